# revision 10
# baseline (speedup 1.0000x reference)
"""MoE (top-2 of 8 experts + shared expert, SwiGLU) Trainium2 Bass kernel.

Contract: kernel(**inputs) takes the FULL inputs of reference.setup_inputs()
(B=4, T=1024, D=512; E=8, TOP_K=2, H=256) and returns the full [B,T,D]
output. Internally the 4096 tokens are sharded 512-per-core across 8
NeuronCores (data parallel); every core holds all expert weights and
computes router -> softmax -> top-2 combine -> per-expert SwiGLU (dense,
masked by the combine weight, folded in before the down projection) plus
the shared expert, accumulating everything for its token shard in PSUM.

Matmuls run as float32r (full PE rate at free dim >= 256); the router path
stays plain float32 so top-2 selection matches the fp32 reference.

Weights are repacked on the host so every SBUF partition receives one
contiguous 4-8KB run per DMA (large-packet DMA, ~2x effective HBM BW vs
the naive 1KB-per-partition layout).
"""

import numpy as np

import concourse.bass as bass
import concourse.tile as tile
from concourse import mybir
from concourse.bass_utils import run_bass_kernel_spmd
from concourse.masks import make_identity
from concourse.vector_clock import ScopedClock

F32 = mybir.dt.float32
F32R = mybir.dt.float32r
AF = mybir.ActivationFunctionType
ALU = mybir.AluOpType
AX = mybir.AxisListType

N_CORES = 8
B, T_SEQ, D = 4, 1024, 512
E, H = 8, 256
N = B * T_SEQ            # 4096 tokens
T = N // N_CORES         # 512 tokens per core
DC = D // 128            # 4 contraction chunks of 128
HC = H // 128            # 2 hidden chunks of 128
TT = T // 128            # 4 token tiles of 128


# ---------------------------------------------------------------------------
# Workarounds for this walrus build: any instruction may carry at most ONE
# sync wait ("Too many sync wait commands"). Split extra waits onto
# same-engine NoOps placed immediately before (identical semantics: the
# engine queue is in-order).
# ---------------------------------------------------------------------------
class SplitDrainTileContext(tile.TileContext):
    def _drain_and_barrier(self, tick_clock, wait_clock):
        vc = tick_clock.global_clock
        for p in range(len(vc)):
            if vc[p] <= 0:
                continue
            nop = self.nc.sync.nop(nofuse=True)
            pc = ScopedClock()
            pc.require_at_least(None, p, vc[p])
            wait_clock.add_sem_waits(nop.ins, pc)
        full = ScopedClock({None: vc})
        drain_inst = self.nc.sync.drain()
        wait_clock.add_sem_waits(drain_inst.ins, full.copy(), full)

        self.nc.all_engine_barrier()
        assert self.sems is not None
        popped = self.nc._tile_sem_poison_stack.pop()
        assert popped is self._sem_poison
        self.nc.clear_and_free_semaphores(list(self.sems.allocated().values()))
        self.nc.all_engine_barrier()


def split_multi_waits(nc):
    n_split = 0
    for f in nc.m.functions:
        for blk in f.blocks:
            new_insts = []
            changed = False
            for ins in blk.instructions:
                si = ins.sync_info
                waits = list(si.on_wait) if si and si.on_wait else []
                if len(waits) > 1:
                    for w in waits[:-1]:
                        nop = mybir.InstNoOp(name=f"I-wsplit-{n_split}")
                        n_split += 1
                        nop.engine = ins.engine
                        nop.sync_info = mybir.SyncInfo(on_wait=[w], on_update=[])
                        new_insts.append(nop)
                    ins.sync_info = mybir.SyncInfo(
                        on_wait=[waits[-1]], on_update=list(si.on_update or [])
                    )
                    changed = True
                new_insts.append(ins)
            if changed:
                blk.instructions = new_insts
    return n_split


# ---------------------------------------------------------------------------
# Kernel build
# ---------------------------------------------------------------------------
def build(split: bool = True, sigmoid_silu: bool = False):
    nc = bass.Bass("TRN2", target_bir_lowering=False, debug=False,
                   num_devices=N_CORES)

    # All layouts give each partition one contiguous run per DMA.
    xT = nc.dram_tensor("xT", [128, DC * T], F32, kind="ExternalInput").ap()
    router = nc.dram_tensor("router", [128, DC * E], F32, kind="ExternalInput").ap()
    wg = nc.dram_tensor("wg", [E, 128, DC * H], F32, kind="ExternalInput").ap()
    wu = nc.dram_tensor("wu", [E, 128, DC * H], F32, kind="ExternalInput").ap()
    wd = nc.dram_tensor("wd", [E, 128, HC * D], F32, kind="ExternalInput").ap()
    sg_w = nc.dram_tensor("sg_w", [128, DC * H], F32, kind="ExternalInput").ap()
    su_w = nc.dram_tensor("su_w", [128, DC * H], F32, kind="ExternalInput").ap()
    sd_w = nc.dram_tensor("sd_w", [128, HC * D], F32, kind="ExternalInput").ap()
    out = nc.dram_tensor("out", [TT, 128, D], F32, kind="ExternalOutput").ap()

    with SplitDrainTileContext(nc) as tc:
        with (
            tc.tile_pool(name="const", bufs=1) as const,
            tc.tile_pool(name="wpool", bufs=1) as wpool,
            tc.tile_pool(name="work", bufs=1) as work,
            tc.tile_pool(name="ps", bufs=1, space="PSUM") as ps,
            tc.tile_pool(name="psy", bufs=1, space="PSUM") as psy,
        ):
            # ---- static loads ------------------------------------------------
            xt = const.tile([128, DC * T], F32)       # router path (exact fp32)
            nc.sync.dma_start(out=xt, in_=xT)
            xtr = const.tile([128, DC * T], F32R)     # expert path
            nc.sync.dma_start(out=xtr, in_=xT.bitcast(F32R))
            rt = const.tile([128, DC * E], F32)
            nc.sync.dma_start(out=rt, in_=router)

            ident = const.tile([128, 128], F32)
            make_identity(nc, ident[:])
            ones_f = const.tile([1, 128], F32)
            nc.vector.memset(ones_f[:], 1.0)
            ones = const.tile([1, 128], F32R)
            nc.scalar.copy(ones[:], ones_f[:])

            # ---- phase 1: router probs -> top-2 combine -> combT -> cbb ------
            # combT_e: combine column e as a [1, T] row at base partition 0
            # (matmul rhs must sit at base partition 0/32/64).
            combT = [const.tile([1, T], F32R, tag=f"combT{e}", name=f"combT{e}")
                     for e in range(E)]
            for tt in range(TT):
                p_ps = ps.tile([128, E], F32, tag="ph1", bufs=2, name="p_ps")
                for c in range(DC):
                    nc.tensor.matmul(
                        p_ps[:], xt[:, c * T + tt * 128:c * T + (tt + 1) * 128],
                        rt[:, c * E:(c + 1) * E], start=(c == 0), stop=(c == DC - 1))
                negm = work.tile([128, 1], F32, tag="negm", bufs=2)
                nc.vector.tensor_reduce(negm[:], p_ps[:], AX.X, ALU.max, negate=True)
                e_sb = work.tile([128, E], F32, tag="e_sb", bufs=2)
                ssum = work.tile([128, 1], F32, tag="ssum", bufs=2)
                nc.scalar.activation(e_sb[:], p_ps[:], AF.Exp, bias=negm[:],
                                     accum_out=ssum[:])
                rs = work.tile([128, 1], F32, tag="rs", bufs=2)
                nc.vector.reciprocal(rs[:], ssum[:])
                probs = work.tile([128, E], F32, tag="probs", bufs=2)
                nc.vector.tensor_scalar_mul(probs[:], e_sb[:], rs[:])
                m1 = work.tile([128, 1], F32, tag="m1", bufs=2)
                nc.vector.tensor_reduce(m1[:], probs[:], AX.X, ALU.max)
                p2 = work.tile([128, E], F32, tag="p2", bufs=2)
                nc.vector.scalar_tensor_tensor(p2[:], probs[:], m1[:], probs[:],
                                               op0=ALU.is_lt, op1=ALU.mult)
                m2 = work.tile([128, 1], F32, tag="m2", bufs=2)
                nc.vector.tensor_reduce(m2[:], p2[:], AX.X, ALU.max)
                comb = work.tile([128, E], F32, tag="comb", bufs=2)
                nc.vector.scalar_tensor_tensor(comb[:], probs[:], m2[:], probs[:],
                                               op0=ALU.is_ge, op1=ALU.mult)
                for e in range(E):
                    ct_ps = ps.tile([1, 128], F32, tag="ph1", bufs=2, name="ct_ps")
                    nc.tensor.transpose(ct_ps[:], comb[:, e:e + 1], ident[:])
                    nc.scalar.copy(combT[e][0:1, tt * 128:(tt + 1) * 128], ct_ps[:])

            # broadcast combine rows across partitions: cbb[e][p, t] = combT[e][0, t]
            cbb = [const.tile([128, T], F32, tag=f"cbb{e}", name=f"cbb{e}")
                   for e in range(E)]
            for e in range(E):
                b_ps = ps.tile([128, T], F32, tag="ph1", bufs=2, name="b_ps")
                nc.tensor.matmul(b_ps[:], ones[:], combT[e][:], start=True, stop=True)
                nc.vector.tensor_copy(cbb[e][:], b_ps[:])

            # ---- phase 2: experts --------------------------------------------
            # yacc[tt] accumulates shared + all 8 weighted experts in PSUM.
            yacc = [psy.tile([128, D], F32, tag=f"yacc{tt}", name=f"yacc{tt}")
                    for tt in range(TT)]
            n_groups = (E + 1) * HC  # accumulation steps per token tile

            def ffn(tag, wg_ap, wu_ap, wd_ap, cbb_tile, step0):
                """One expert (or the shared expert when cbb_tile is None)."""
                wgt = wpool.tile([128, DC * H], F32R, tag="wg", bufs=3,
                                 name=f"{tag}wg")
                wut = wpool.tile([128, DC * H], F32R, tag="wu", bufs=3,
                                 name=f"{tag}wu")
                wdt = wpool.tile([128, HC * D], F32R, tag="wd", bufs=3,
                                 name=f"{tag}wd")
                nc.sync.dma_start(out=wgt, in_=wg_ap.bitcast(F32R))
                nc.sync.dma_start(out=wut, in_=wu_ap.bitcast(F32R))
                nc.sync.dma_start(out=wdt, in_=wd_ap.bitcast(F32R))

                for h in range(HC):
                    g_ps = ps.tile([128, T], F32, tag="g_ps", bufs=1, name="g_ps")
                    u_ps = ps.tile([128, T], F32, tag="u_ps", bufs=1, name="u_ps")
                    for c in range(DC):
                        nc.tensor.matmul(
                            g_ps[:], wgt[:, c * H + h * 128:c * H + (h + 1) * 128],
                            xtr[:, c * T:(c + 1) * T],
                            start=(c == 0), stop=(c == DC - 1))
                    for c in range(DC):
                        nc.tensor.matmul(
                            u_ps[:], wut[:, c * H + h * 128:c * H + (h + 1) * 128],
                            xtr[:, c * T:(c + 1) * T],
                            start=(c == 0), stop=(c == DC - 1))
                    sg = work.tile([128, T], F32, tag="sg", bufs=2)
                    if sigmoid_silu:
                        # CoreSim lacks Silu; x*sigmoid(x) is identical math.
                        sgm = work.tile([128, T], F32, tag="sgm", bufs=2)
                        nc.scalar.activation(sgm[:], g_ps[:], AF.Sigmoid)
                        nc.vector.tensor_mul(sg[:], sgm[:], g_ps[:])
                    else:
                        nc.scalar.activation(sg[:], g_ps[:], AF.Silu)
                    hid = work.tile([128, T], F32R, tag="hid", bufs=3)
                    if cbb_tile is not None:
                        uw = work.tile([128, T], F32, tag="uw", bufs=2)
                        nc.vector.tensor_mul(uw[:], u_ps[:], cbb_tile[:])
                        nc.vector.tensor_mul(hid[:], sg[:], uw[:])
                    else:
                        nc.vector.tensor_mul(hid[:], sg[:], u_ps[:])
                    step = step0 + h
                    for tt in range(TT):
                        nc.tensor.matmul(
                            yacc[tt][:], hid[:, tt * 128:(tt + 1) * 128],
                            wdt[:, h * D:(h + 1) * D],
                            start=(step == 0), stop=(step == n_groups - 1))

            ffn("s", sg_w, su_w, sd_w, None, 0)
            for e in range(E):
                ffn(f"e{e}", wg[e], wu[e], wd[e], cbb[e], (1 + e) * HC)

            # ---- phase 3: writeback ------------------------------------------
            for tt in range(TT):
                o_sb = work.tile([128, D], F32, tag="o_sb", bufs=2)
                nc.scalar.copy(o_sb[:], yacc[tt][:])
                nc.sync.dma_start(out=out[tt], in_=o_sb[:])

    if split:
        split_multi_waits(nc)
    return nc


_CACHE = {}


def _get_nc():
    if "nc" not in _CACHE:
        _CACHE["nc"] = build()
    return _CACHE["nc"]


def _pack_k(w):
    """[(c*128), M] -> [128, c*M]: partition p, col c*M+m = w[c*128+p, m]."""
    K, M = w.shape
    c = K // 128
    return np.ascontiguousarray(
        w.reshape(c, 128, M).transpose(1, 0, 2).reshape(128, c * M))


def _in_maps(x, router, shared_gate, shared_up, shared_down, W_gate, W_up, W_down):
    x_flat = np.ascontiguousarray(np.asarray(x, dtype=np.float32)).reshape(N, D)
    router = np.asarray(router, dtype=np.float32)
    W_gate = np.asarray(W_gate, dtype=np.float32)
    W_up = np.asarray(W_up, dtype=np.float32)
    W_down = np.asarray(W_down, dtype=np.float32)
    shared_gate = np.asarray(shared_gate, dtype=np.float32)
    shared_up = np.asarray(shared_up, dtype=np.float32)
    shared_down = np.asarray(shared_down, dtype=np.float32)

    common = {
        "router": _pack_k(router),
        "wg": np.stack([_pack_k(W_gate[e]) for e in range(E)]),
        "wu": np.stack([_pack_k(W_up[e]) for e in range(E)]),
        "wd": np.stack([_pack_k(W_down[e]) for e in range(E)]),
        "sg_w": _pack_k(shared_gate),
        "su_w": _pack_k(shared_up),
        "sd_w": _pack_k(shared_down),
    }
    maps = []
    for c in range(N_CORES):
        shard = x_flat[c * T:(c + 1) * T, :]            # [T, D]
        maps.append({"xT": _pack_k(np.ascontiguousarray(shard.T)), **common})
    return maps


def run(inputs: dict, trace: bool = False):
    """Run the SPMD kernel; returns (full_output, BassKernelResults)."""
    nc = _get_nc()
    maps = _in_maps(**inputs)
    res = run_bass_kernel_spmd(nc, maps, list(range(N_CORES)), trace=trace)
    parts = [res.results[c]["out"].reshape(T, D) for c in range(N_CORES)]
    full = np.concatenate(parts, axis=0).reshape(B, T_SEQ, D)
    return full, res


def kernel(**inputs) -> np.ndarray:
    full, _ = run(inputs, trace=False)
    return full
